# revision 28
# baseline (speedup 1.0000x reference)
"""BatchHardTriplet loss kernel for Trainium2 (8 NeuronCores, SPMD).

Strategy
--------
Rows are sorted by label on the host; each core owns 1024 rows (8 chunks of
128) and computes its [128, 8192] sim block per chunk against an all-gathered,
per-core column-rotated operand so all positives land in window cols [0,1152).
The eq-mask (-2 where labels match, incl. diagonal) is accumulated into the
window region on the TensorEngine via an identity @ mask_fp8 matmul.

Drain design: the kernel iterates column-block-major (all 8 row-chunks
consume each 1024-col block of embA in DMA-arrival order, so the first
chunks are not input-starved). Each [128,1024] PSUM tile is retired by one
engine, alternating:
  - VectorE: exact tensor_reduce(max) on half the tiles, plus the per-chunk
    window min (hardest positive) over cols [128*mc, 128*mc+512).
  - ScalarE: activation(Exp, scale=80) with the fused SUM accumulator:
    one 1x pass yields sum(exp(80*sim)) per row; ln(sum)/80 is a sharp
    softmax upper bound on the tile max (bias ~+0.004 vs the 2e-2 loss
    tolerance), and the -2 mask makes window positives vanish (exp(-160)).
    No fp16 copies, no max trees, no final reduces.
The host combines exact maxes with the LSE partials, applies validity and
the final relu/mean (labels-only logic).
"""

import os
import sys
import numpy as np

sys.path.insert(0, "/opt/trn_rl_repo")

B = 8192
D = 128
M = 8            # cores
R = B // M       # 1024 rows per core
MC = R // 128    # 8 chunks of 128 rows per core
MARGIN = 0.3
MASKV = -2.0     # mask value added to label-equal sims
TAU = 80.0       # LSE sharpness (exp(80*sim) <= e^80 < f32 max)
DELTA = 192      # rotation offset: chunk windows at [128*mc, 128*mc+512)
NS = 10          # output slots per chunk

_CACHE = {}


def _build_program():
    """Build (once) the Bass program shared by all 8 cores."""
    if "nc" in _CACHE:
        return _CACHE["nc"]

    import concourse.bass as bass
    import concourse.bacc as bacc
    import concourse.mybir as mybir
    from concourse import tile

    f32 = mybir.dt.float32
    bf16 = mybir.dt.bfloat16
    fp8 = mybir.dt.float8e4
    Exp = mybir.ActivationFunctionType.Exp
    MAX = mybir.AluOpType.max
    MIN = mybir.AluOpType.min
    X = mybir.AxisListType.X

    nc = bacc.Bacc(None, target_bir_lowering=False)

    embA = nc.dram_tensor("embA", [D, B], bf16, kind="ExternalInput")
    embB = nc.dram_tensor("embB", [D, R], bf16, kind="ExternalInput")
    masks = nc.dram_tensor("masks", [128, MC, 512], fp8, kind="ExternalInput")
    iden = nc.dram_tensor("iden", [128, 128], fp8, kind="ExternalInput")
    outs = nc.dram_tensor("outs", [128, MC, NS], f32, kind="ExternalOutput")

    with tile.TileContext(nc) as tc:
        with (
            tc.tile_pool(name="big", bufs=1) as big,
            tc.tile_pool(name="ps", bufs=4, space="PSUM") as ps,
            tc.tile_pool(name="jk", bufs=3) as jk,
            tc.tile_pool(name="st", bufs=1) as st,
        ):
            # DMA order: first-matmul operands land first. Spread issue cost
            # over two queues (sync + gpsimd).
            Bt = big.tile([D, R], bf16)
            nc.sync.dma_start(Bt[:, 0:128], embB[:, 0:128])
            A = big.tile([D, B], bf16)
            nc.sync.dma_start(A[:, 0:512], embA[:, 0:512])
            nc.sync.dma_start(Bt[:, 128:R], embB[:, 128:R])
            Id = big.tile([128, 128], fp8)
            nc.scalar.dma_start(Id[:], iden[:])
            Mk = big.tile([128, MC, 512], fp8)
            nc.scalar.dma_start(Mk[:, 0:2, :], masks[:, 0:2, :])
            nc.sync.dma_start(A[:, 512:1024], embA[:, 512:1024])
            nc.scalar.dma_start(Mk[:, 2:MC, :], masks[:, 2:MC, :])
            for cb in range(1, 8):
                nc.sync.dma_start(A[:, cb * 1024:(cb + 1) * 1024],
                                  embA[:, cb * 1024:(cb + 1) * 1024])

            out_t = st.tile([128, MC, NS], f32)
            nc.vector.memset(out_t[:], 10.0)

            # column-block-major: consume embA in arrival order so the
            # first chunks aren't input-DMA starved
            n_mx = [0] * MC
            n_se = [0] * MC
            for cb in range(8):
                for mc in range(MC):
                    lhsT = Bt[:, mc * 128:(mc + 1) * 128]
                    wb = 128 * mc  # chunk window = cols [wb, wb+512)
                    P = ps.tile([128, 1024], f32, tag="psum",
                                name=f"P{mc}_{cb}")
                    for t in range(2):
                        lo = cb * 1024 + t * 512
                        nc.tensor.matmul(
                            P[:, t * 512:(t + 1) * 512],
                            lhsT,
                            A[:, lo:lo + 512],
                            start=True,
                            stop=True,
                        )
                    # accumulate -2*eq mask into this chunk's window cols,
                    # split at PSUM bank (512-col) boundaries
                    def mask_pieces(p_lo, p_hi, m_off):
                        while p_lo < p_hi:
                            p_end = min((p_lo // 512 + 1) * 512, p_hi)
                            nc.tensor.matmul(
                                P[:, p_lo:p_end], Id[:],
                                Mk[:, mc, m_off:m_off + (p_end - p_lo)],
                                start=False, stop=True,
                                skip_group_check=True,
                            )
                            m_off += p_end - p_lo
                            p_lo = p_end
                    if cb == 0:
                        mask_pieces(wb, min(wb + 512, 1024), 0)
                    elif cb == 1 and wb + 512 > 1024:
                        mask_pieces(0, wb - 512, 512 - (wb - 512))
                    if cb == 0:
                        # hardest-positive: min over the narrow true window
                        # [wb+112, wb+400) (asserted at prep; T0 part here)
                        nc.vector.tensor_reduce(
                            out_t[:, mc, 0:1],
                            P[:, wb + 112:min(wb + 400, 1024)],
                            axis=X, op=MIN)
                    if cb == 1 and wb + 400 > 1024:
                        nc.vector.tensor_reduce(
                            out_t[:, mc, 1:2], P[:, 0:wb - 624],
                            axis=X, op=MIN)
                    if (cb + mc) % 2 == 0:
                        # ScalarE LSE retires the tile in one pass (masked
                        # window positives vanish: exp(80(s-2)) ~= 0)
                        j = jk.tile([128, 1024], bf16, tag="jk",
                                    name=f"j{mc}_{cb}")
                        nc.scalar.activation(
                            j[:], P[:], Exp, scale=TAU,
                            accum_out=out_t[:, mc, 6 + n_se[mc]:
                                            7 + n_se[mc]])
                        n_se[mc] += 1
                    else:
                        # exact max on DVE
                        nc.vector.tensor_reduce(
                            out_t[:, mc, 2 + n_mx[mc]: 3 + n_mx[mc]], P[:],
                            axis=X, op=MAX)
                        n_mx[mc] += 1

            nc.sync.dma_start(outs[:], out_t[:])

    nc.compile()
    _CACHE["nc"] = nc
    return nc


def _prep_inputs(emb, labels):
    """Sort by label, build per-core permuted operands + fp8 masks."""
    import ml_dtypes

    emb = np.asarray(emb, dtype=np.float32)
    labels = np.asarray(labels)
    order = np.argsort(labels, kind="stable")
    labs = labels[order]
    embs = emb[order]
    embT = np.ascontiguousarray(embs.T)  # [D, B]

    starts = np.searchsorted(labs, labs, side="left")
    ends = np.searchsorted(labs, labs, side="right")
    counts = ends - starts
    valid = (counts >= 2) & (counts < B)

    iden = np.eye(128, dtype=ml_dtypes.float8_e4m3)

    in_maps = []
    for c in range(M):
        r0 = c * R
        s = int(starts[r0])
        for mc in range(MC):
            rr0 = r0 + mc * 128
            lo = int(starts[rr0]) - s + DELTA
            hi = int(ends[rr0 + 127]) - s + DELTA
            assert 128 * mc + 112 <= lo and hi <= 128 * mc + 400, (
                f"chunk window [{lo},{hi}) outside "
                f"[{128*mc+112},{128*mc+400})"
            )
        # rotate so chunk mc's positives land in cols [128*mc, 128*mc+512)
        perm = (s - DELTA + np.arange(B)) % B
        embA = np.ascontiguousarray(embT[:, perm]).astype(ml_dtypes.bfloat16)
        embB = np.ascontiguousarray(embT[:, r0:r0 + R]).astype(ml_dtypes.bfloat16)
        lab_rows = labs[r0:r0 + R].reshape(MC, 128)
        # per-chunk 512-col windows at [128*mc, 128*mc+512)
        win_cols = (128 * np.arange(MC)[:, None] + np.arange(512)[None, :])
        lab_win = labs[perm[win_cols]]                        # [MC, 512]
        eq = lab_rows[:, :, None] == lab_win[:, None, :]      # [MC, 128, 512]
        masks = np.where(eq, np.float32(MASKV), np.float32(0.0)).astype(
            ml_dtypes.float8_e4m3
        )
        # device layout [128 partitions, MC, 512]
        masks = np.ascontiguousarray(masks.transpose(1, 0, 2))
        in_maps.append(
            {"embA": embA, "embB": embB, "masks": masks, "iden": iden}
        )
    return in_maps, valid


def _postprocess(results, valid):
    minv = np.zeros(B, dtype=np.float32)
    maxv = np.zeros(B, dtype=np.float32)
    for c, res in enumerate(results):
        o = res["outs"].astype(np.float64)  # [128, MC, NS]
        for mc in range(MC):
            mn = o[:, mc, 0]
            if 128 * mc + 400 > 1024:
                mn = np.minimum(mn, o[:, mc, 1])
            mx = o[:, mc, 2:6].max(axis=1)
            se = o[:, mc, 6:10].sum(axis=1)
            lse = np.log(np.maximum(se, 1e-300)) / TAU
            hn_sim = np.maximum(mx, lse)
            rows = slice(c * R + mc * 128, c * R + mc * 128 + 128)
            minv[rows] = mn
            maxv[rows] = hn_sim
    hp = 1.0 - (minv - MASKV)  # hardest positive distance (undo mask)
    hn = 1.0 - maxv            # hardest negative distance
    per_row = np.maximum(0.0, hp - hn + MARGIN)
    cnt = int(valid.sum())
    if cnt == 0:
        return np.float32(0.0)
    return np.float32(np.sum(per_row[valid]) / cnt)


def run_device(in_maps, trace=False):
    from concourse.bass_utils import run_bass_kernel_spmd

    nc = _build_program()
    return run_bass_kernel_spmd(nc, in_maps, list(range(M)), trace=trace)


def kernel(emb, labels):
    in_maps, valid = _prep_inputs(emb, labels)
    out = run_device(in_maps, trace=False)
    return _postprocess(out.results, valid)


if __name__ == "__main__":
    # smoke test with random data
    rng = np.random.default_rng(0)
    emb = rng.standard_normal((B, D)).astype(np.float32)
    emb /= np.linalg.norm(emb, axis=1, keepdims=True) + 1e-12
    labels = rng.integers(0, 512, B).astype(np.int32)
    print(kernel(emb, labels))
